# revision 19
# baseline (speedup 1.0000x reference)
"""GRU autoencoder Trainium2 kernel (bf16, seedless/no-inject redesign).

Data-parallel over batch: 8 cores x 64 rows. All gate matmuls are bf16
column-tile duos (tile_position (0,0)/(0,64)): each [128,512] PSUM tile
holds both 512-col halves of one gate. PSUM bias seeds, the PE inject,
and the z/n transposes of the old design are gone:
  - biases are added by DVE tensor_tensor (PSUM + stacked-bias tile),
  - n-preact sum (in + r*hn) runs on DVE,
  - the h-update runs in NORMAL space (h' = n + z*(h-n)) on 16-bit DVE
    ops, and only h' is transposed back (8 row-paired N=64 matmuls).
Decoder emits the z-gate last so the exposed post-matmul chain is just
zpre-add -> sigmoid -> e=z*d -> h'=n+e; zout duos fill the PE during it.
Hidden state is carried in both layouts: h_norm (stacked [128,512]) and
hT (hT[klo, 64*khi+b] = h[b, 128*khi+klo]).
"""
import os
import sys
import types

import ml_dtypes
import numpy as np

import concourse.bass as bass
import concourse.mybir as mybir
import concourse.tile as tile
from concourse import bass_utils

F32 = mybir.dt.float32
BF16 = mybir.dt.bfloat16
AF = mybir.ActivationFunctionType
OP = mybir.AluOpType

N_CORES = 8
B, T, I, H = 512, 128, 512, 1024
BL = B // N_CORES  # 64


# ---------------------------------------------------------------- fixups
def _split_multi_waits(nc, max_waits=1):
    """This walrus build allows only one sync-wait per instruction; hoist
    excess waits onto preceding NoOps (same engine, so semantics hold)."""
    for f in nc.m.functions:
        for blk in f.blocks:
            insts = blk.instructions
            if not any(
                i.sync_info is not None
                and i.sync_info.on_wait
                and len(i.sync_info.on_wait) > max_waits
                for i in insts
            ):
                continue
            new = []
            for inst in insts:
                si = inst.sync_info
                if si is not None and si.on_wait and len(si.on_wait) > max_waits:
                    waits = list(si.on_wait)
                    extra, keep = waits[:-max_waits], waits[-max_waits:]
                    for cs in range(0, len(extra), max_waits):
                        nop = mybir.InstNoOp(
                            name=nc.get_next_instruction_name(),
                            engine=inst.engine,
                            ins=[],
                            outs=[],
                            sync_info=mybir.SyncInfo(
                                on_wait=extra[cs : cs + max_waits], on_update=[]
                            ),
                        )
                        nc.register_instruction(nop)
                        new.append(nop)
                    si.on_wait = keep
                new.append(inst)
            insts[:] = new


def _install_ntff_hook():
    if "antenv.axon_hooks" in sys.modules:
        return True
    mod = types.ModuleType("antenv.axon_hooks")
    state = {"hook": None}
    mod.set_axon_ntff_profile_hook = lambda h: state.__setitem__("hook", h)
    mod.get_axon_ntff_profile_hook = lambda: state["hook"]
    sys.modules["antenv.axon_hooks"] = mod
    try:
        import antenv

        antenv.axon_hooks = mod
        from trn_agent_boot.trn_boot import _ntff_profile_via_ctypes

        hook = _ntff_profile_via_ctypes("/opt/axon/libaxon_pjrt.so")
        if hook is None:
            return False
        mod.set_axon_ntff_profile_hook(hook)
        return True
    except Exception:
        return False


# ---------------------------------------------------------------- program
def build_nc(n_steps=T):
    nc = bass.Bass("TRN2", target_bir_lowering=False, debug=False, num_devices=N_CORES)

    xT_d = nc.dram_tensor("xT", [n_steps, 4, 128, BL], BF16, kind="ExternalInput").ap()
    wih_d = nc.dram_tensor("wihT", [4, 128, 3 * H], BF16, kind="ExternalInput").ap()
    whh_d = nc.dram_tensor("whhT", [8, 128, 3 * H], BF16, kind="ExternalInput").ap()
    wcb_d = nc.dram_tensor("wcombT", [8, 128, 4 * H], BF16, kind="ExternalInput").ap()
    wz_d = nc.dram_tensor("wzT", [8, 128, I], BF16, kind="ExternalInput").ap()
    be_d = nc.dram_tensor("bias_enc", [128, 4, 512], BF16, kind="ExternalInput").ap()
    bd_d = nc.dram_tensor("bias_dec", [128, 4, 512], BF16, kind="ExternalInput").ap()
    bz_d = nc.dram_tensor("bz_rep", [128, I], BF16, kind="ExternalInput").ap()
    id_d = nc.dram_tensor("iden2", [128, 64], BF16, kind="ExternalInput").ap()
    ss_d = nc.dram_tensor("sstat", [128, 64], BF16, kind="ExternalInput").ap()
    bhe_d = nc.dram_tensor("behn_rep", [128, H], BF16, kind="ExternalInput").ap()
    bdz_d = nc.dram_tensor("bdz_rep", [128, H], BF16, kind="ExternalInput").ap()
    h0_d = nc.dram_tensor("h0T", [128, 512], BF16, kind="ExternalInput").ap()
    z_d = nc.dram_tensor("z", [BL, n_steps, I], F32, kind="ExternalOutput").ap()

    # gate -> column offset (PyTorch order r,z,n); bias slot index matches
    C0 = {"r": 0, "z": H, "in": 2 * H, "hn": 3 * H}
    BSLOT = {"r": 0, "z": 1, "in": 2, "hn": 3}

    with tile.TileContext(nc) as tc:
        with (
            tc.tile_pool(name="wgt", bufs=1) as wgt,
            tc.tile_pool(name="cst", bufs=1) as cst,
            tc.tile_pool(name="hst", bufs=5) as hst,
            tc.tile_pool(name="hnm", bufs=3) as hnm,
            tc.tile_pool(name="xts", bufs=4) as xts,
            tc.tile_pool(name="gsb", bufs=2) as gsb,
            tc.tile_pool(name="tmp", bufs=2) as tmpp,
            tc.tile_pool(name="zo", bufs=2) as zop,
            tc.tile_pool(name="ps", bufs=8, space="PSUM") as ps,
        ):
            iden2 = cst.tile([128, 64], BF16)
            nc.sync.dma_start(iden2[:], id_d[:])
            sstat = cst.tile([128, 64], BF16)
            nc.sync.dma_start(sstat[:], ss_d[:])
            bias_enc = cst.tile([128, 4, 512], BF16)
            nc.sync.dma_start(bias_enc[:], be_d[:])
            behn_rep = cst.tile([128, H], BF16)
            nc.sync.dma_start(behn_rep[:], bhe_d[:])
            bias_dec = cst.tile([128, 4, 512], BF16)
            bdz_rep = cst.tile([128, H], BF16)
            bz_rep = cst.tile([128, I], BF16)
            nc.sync.dma_start(bz_rep[:], bz_d[:])
            hT = hst.tile([128, 512], BF16, tag="h")
            nc.sync.dma_start(hT[:], h0_d[:])
            hN = hnm.tile([128, 512], BF16, tag="hn")
            nc.sync.dma_start(hN[:], h0_d[:])

            # encoder-critical DMAs first; decoder weights follow and overlap
            # with encoder compute
            xt_tiles = {}
            wih = wgt.tile([128, 4, 3 * H], BF16)
            for k in range(4):
                nc.sync.dma_start(wih[:, k, :], wih_d[k])
            for t in range(min(3, n_steps)):
                xt_tiles[t] = xts.tile([128, 4, BL], BF16, tag="x", name=f"xt{t}")
                for k in range(4):
                    nc.sync.dma_start(xt_tiles[t][:, k, :], xT_d[t, k])
            whh = wgt.tile([128, 8, 3 * H], BF16)
            for k in range(8):
                eng = nc.sync if k % 2 == 0 else nc.scalar
                eng.dma_start(whh[:, k, :], whh_d[k])
            nc.scalar.dma_start(bias_dec[:], bd_d[:])
            nc.scalar.dma_start(bdz_rep[:], bdz_d[:])
            wcb = wgt.tile([128, 8, 4 * H], BF16)
            for k in range(8):
                nc.scalar.dma_start(wcb[:, k, :], wcb_d[k])
            wz = wgt.tile([128, 8, I], BF16)
            for k in range(8):
                nc.scalar.dma_start(wz[:, k, :], wz_d[k])

            def dup(t, stat, mov_lo, mov_hi, start, stop):
                """One column-tile duo: two concurrent M=64 matmuls sharing
                the stationary operand."""
                nc.tensor.matmul(t[0:64, :], stat, mov_lo, start=start,
                                 stop=stop, tile_position=(0, 0),
                                 skip_group_check=True)
                nc.tensor.matmul(t[64:128, :], stat, mov_hi, start=start,
                                 stop=stop, tile_position=(0, 64),
                                 skip_group_check=True)

            def emit_gi(g, xt, gates, ks=range(4), start=False):
                for gate in gates:
                    c0 = C0[gate]
                    t = g[gate]
                    for k in ks:
                        dup(t, xt[:, k, :], wih[:, k, c0 : c0 + 512],
                            wih[:, k, c0 + 512 : c0 + 1024],
                            start=(start and k == 0), stop=False)

            # k-order matches hT'-sub completion: sub1 covers hT cols
            # 0-255 (khi 0-3)... but transpose halves land as (j01 -> khi
            # {0,1,4,5}); with the two subs covering cols 0-255 / 256-511,
            # (0,1,2,3,...) waits sub1 then sub2.  Keep natural order.
            # k-order follows hT'-sub completion: half A of the w-
            # transpose covers hT cols 0-127 & 256-383 (k 0,1,4,5), half B
            # the rest -- so gh starts as soon as half A lands.
            GH_KS = (0, 1, 4, 5, 2, 3, 6, 7)

            def emit_gh(g, w, gates, hTc, start=False, stop=True, c0map=None):
                for gate in gates:
                    c0 = (c0map or C0)[gate]
                    t = g[gate]
                    for i, k in enumerate(GH_KS):
                        hs = hTc[:, 64 * k : 64 * k + 64]
                        dup(t, hs, w[:, k, c0 : c0 + 512],
                            w[:, k, c0 + 512 : c0 + 1024],
                            start=(start and i == 0), stop=(stop and i == 7))

            def seed(t, rep, start=True):
                dup(t, sstat[:, :], rep[:, 0:512], rep[:, 512:1024],
                    start=start, stop=False)

            def transpose_h(src_sb, pT0, pT8):
                """Row-paired transpose of stacked h' [128,512]: pT0 gets
                h-dims 0-511 (hT cols 0-255 layout), pT8 the hi half. Two
                separate PSUM banks: the row-paired matmuls write the same
                partitions concurrently, so they must not share a bank."""
                for j in range(4):
                    nc.tensor.matmul(
                        pT0[:, 64 * j : 64 * j + 64],
                        src_sb[0:64, 128 * j : 128 * j + 128],
                        iden2[0:64, :], start=True, stop=True,
                        tile_position=(0, 0), skip_group_check=True,
                    )
                    nc.tensor.matmul(
                        pT8[:, 64 * j : 64 * j + 64],
                        src_sb[64:128, 128 * j : 128 * j + 128],
                        iden2[64:128, :], start=True, stop=True,
                        tile_position=(64, 0), skip_group_check=True,
                    )

            def new_gates(t_id, gates):
                g = {}
                for gate in gates:
                    g[gate] = ps.tile([128, 512], F32, tag="ps", bufs=5,
                                      name=f"p{gate}{t_id}")
                return g

            def step_tail(t_id, g, bias, dec_order):
                """DVE/ACT tail with the w-update: w = (z-1)*(n-h), so
                h' = h - w in both layouts.  The n/z chain after the last
                gate is split into column halves so transposes and hT-subs
                pipeline with it."""
                rpre = tmpp.tile([128, 512], BF16, tag="rpre", name=f"rp{t_id}")
                rhn = tmpp.tile([128, 512], BF16, tag="rhn", name=f"rh{t_id}")
                inb = tmpp.tile([128, 512], BF16, tag="inb", name=f"ib{t_id}")
                npre = tmpp.tile([128, 512], BF16, tag="npre", name=f"np{t_id}")
                v = tmpp.tile([128, 512], BF16, tag="v", name=f"v{t_id}")
                w = tmpp.tile([128, 512], BF16, tag="w", name=f"w{t_id}")
                z_sb = gsb.tile([128, 512], BF16, tag="z", name=f"z{t_id}")
                r_sb = gsb.tile([128, 512], BF16, tag="r", name=f"r{t_id}")
                n_sb = gsb.tile([128, 512], BF16, tag="n", name=f"n{t_id}")
                LO, HI = slice(0, 256), slice(256, 512)

                def badd(out, gate):
                    s = BSLOT[gate]
                    nc.vector.tensor_add(out[:], g[gate][:], bias[:, s, :])

                if not dec_order:
                    # encoder: z,r early; hn seeded -> rhn reads PSUM direct
                    zpre = tmpp.tile([128, 512], BF16, tag="zpre",
                                     name=f"zp{t_id}")
                    badd(zpre, "z")
                    nc.scalar.activation(z_sb[:], zpre[:], AF.Sigmoid)
                    badd(rpre, "r")
                    nc.scalar.activation(r_sb[:], rpre[:], AF.Sigmoid)
                    badd(inb, "in")
                    nc.vector.tensor_mul(rhn[:], r_sb[:], g["hn"][:])
                    nc.vector.tensor_add(npre[:], inb[:], rhn[:])
                    for s in (LO, HI):
                        nc.scalar.activation(n_sb[:, s], npre[:, s], AF.Tanh)
                    for s in (LO, HI):
                        nc.vector.tensor_sub(v[:, s], n_sb[:, s], hN[:, s])
                        nc.vector.scalar_tensor_tensor(
                            w[:, s], z_sb[:, s], 1.0, v[:, s],
                            OP.subtract, OP.mult)
                else:
                    # decoder: r,hn,in done; z (seeded) completes last and
                    # its sigmoid reads PSUM directly, by halves
                    hnb = tmpp.tile([128, 512], BF16, tag="hnb",
                                    name=f"hb{t_id}")
                    badd(rpre, "r")
                    nc.scalar.activation(r_sb[:], rpre[:], AF.Sigmoid)
                    badd(hnb, "hn")
                    nc.vector.tensor_mul(rhn[:], r_sb[:], hnb[:])
                    badd(inb, "in")
                    nc.vector.tensor_add(npre[:], inb[:], rhn[:])
                    for s in (LO, HI):
                        nc.scalar.activation(n_sb[:, s], npre[:, s], AF.Tanh)
                    for s in (LO, HI):
                        nc.vector.tensor_sub(v[:, s], n_sb[:, s], hN[:, s])
                    for s in (LO, HI):
                        nc.scalar.activation(z_sb[:, s], g["z"][:, s],
                                             AF.Sigmoid)
                    for s in (LO, HI):
                        nc.vector.scalar_tensor_tensor(
                            w[:, s], z_sb[:, s], 1.0, v[:, s],
                            OP.subtract, OP.mult)
                return w

            def finish_h(t_id, w):
                """Per-half: transpose w (row-paired matmul duos), then
                hT' = hT - wT via DVE subs reading the transpose PSUM, so
                gh k(0,1,4,5) unblocks after half A.  h'_norm = h_norm - w."""
                nonlocal hT, hN
                pT0 = ps.tile([128, 512], F32, tag="pt0", bufs=1,
                              name=f"pT0_{t_id}")
                pT8 = ps.tile([128, 512], F32, tag="pt8", bufs=1,
                              name=f"pT8_{t_id}")
                hT_new = hst.tile([128, 512], BF16, tag="h", name=f"h{t_id}")
                for half in (0, 1):
                    for j in (2 * half, 2 * half + 1):
                        nc.tensor.matmul(
                            pT0[:, 64 * j : 64 * j + 64],
                            w[0:64, 128 * j : 128 * j + 128],
                            iden2[0:64, :], start=True, stop=True,
                            tile_position=(0, 0), skip_group_check=True,
                        )
                        nc.tensor.matmul(
                            pT8[:, 64 * j : 64 * j + 64],
                            w[64:128, 128 * j : 128 * j + 128],
                            iden2[64:128, :], start=True, stop=True,
                            tile_position=(64, 0), skip_group_check=True,
                        )
                    s = slice(128 * half, 128 * half + 128)
                    nc.vector.tensor_sub(hT_new[:, s], hT[:, s], pT0[:, s])
                    s2 = slice(256 + 128 * half, 256 + 128 * half + 128)
                    nc.vector.tensor_sub(hT_new[:, s2], hT[:, s2], pT8[:, s])
                hN_new = hnm.tile([128, 512], BF16, tag="hn", name=f"hN{t_id}")
                nc.vector.tensor_sub(hN_new[:], hN[:], w[:])
                hT = hT_new
                hN = hN_new
                return hT_new

            # ================= encoder =================
            cur = new_gates(0, ("z", "r", "in"))
            emit_gi(cur, xt_tiles[0], ("z", "r", "in"), start=True)

            for t in range(n_steps):
                if t + 3 < n_steps:
                    xt_tiles[t + 3] = xts.tile([128, 4, BL], BF16, tag="x",
                                               name=f"xt{t+3}")
                    for k in range(4):
                        nc.sync.dma_start(xt_tiles[t + 3][:, k, :], xT_d[t + 3, k])
                cur["hn"] = ps.tile([128, 512], F32, tag="ps", bufs=5,
                                    name=f"phn{t}")
                seed(cur["hn"], behn_rep)
                emit_gh(cur, whh, ("z", "r"), hT)
                emit_gh(cur, whh, ("hn",), hT, c0map={"hn": 2 * H})
                g = cur
                w = step_tail(t, g, bias_enc, dec_order=False)
                if t + 1 < n_steps:
                    cur = new_gates(t + 1, ("z", "r", "in"))
                    emit_gi(cur, xt_tiles[t + 1], ("z", "r", "in"), start=True)
                finish_h(t, w)
                xt_tiles.pop(t, None)

            # ================= decoder =================
            # z-output pair p covers steps (p, p+1) using entries[p]=H_p and
            # entries[p+1]; its 8 duos spread over steps p+2, p+3 as PE
            # fillers during the exposed tail.
            assert n_steps == 1 or n_steps % 2 == 0
            zfill_ps = [None]
            entries = {}

            def emit_zpair(p, ks, finish):
                hA, hB = entries[p], entries[p + 1]
                if zfill_ps[0] is None:
                    zfill_ps[0] = ps.tile([128, 512], F32, tag="zo", bufs=1,
                                          name=f"pzo{p}")
                tz = zfill_ps[0]
                for k in ks:
                    nc.tensor.matmul(tz[0:64, :], hA[:, 64 * k : 64 * k + 64],
                                     wz[:, k, :], start=(k == 0),
                                     stop=(k == 7), tile_position=(0, 0),
                                     skip_group_check=True)
                    nc.tensor.matmul(tz[64:128, :], hB[:, 64 * k : 64 * k + 64],
                                     wz[:, k, :], start=(k == 0),
                                     stop=(k == 7), tile_position=(0, 64),
                                     skip_group_check=True)
                if finish:
                    zo_sb = zop.tile([128, 512], F32, tag="zo", name=f"zo{p}")
                    nc.vector.tensor_add(zo_sb[:], tz[:], bz_rep[:])
                    nc.sync.dma_start(z_d[:, p, :], zo_sb[0:64, :])
                    nc.sync.dma_start(z_d[:, p + 1, :], zo_sb[64:128, :])
                    zfill_ps[0] = None

            for t in range(n_steps):
                entries[t - 1] = hT  # hT entering step t is H_{t-1}
                cur = new_gates(1000 + t, ("r", "hn", "in", "z"))
                seed(cur["z"], bdz_rep)
                emit_gh(cur, wcb, ("r", "hn", "in"), hT, start=True)
                emit_gh(cur, wcb, ("z",), hT)
                g = cur
                w = step_tail(1000 + t, g, bias_dec, dec_order=True)
                # zout fillers run while the z-chain drains
                if t >= 2 and t % 2 == 0:
                    emit_zpair(t - 2, ks=range(0, 4), finish=False)
                elif t >= 3 and t % 2 == 1:
                    emit_zpair(t - 3, ks=range(4, 8), finish=True)
                finish_h(1000 + t, w)
            entries[n_steps - 1] = hT

            if n_steps >= 2:
                emit_zpair(n_steps - 2, ks=range(0, 8), finish=True)
            elif n_steps == 1:
                # single step: both column-duo halves compute H_0 @ Wz
                tz = ps.tile([128, 512], F32, tag="zo", bufs=1, name="pzo0")
                for k in range(8):
                    dup(tz, entries[0][:, 64 * k : 64 * k + 64], wz[:, k, :],
                        wz[:, k, :], start=(k == 0), stop=(k == 7))
                zo_sb = zop.tile([128, 512], F32, tag="zo", name="zo0")
                nc.vector.tensor_add(zo_sb[:], tz[:], bz_rep[:])
                nc.sync.dma_start(z_d[:, 0, :], zo_sb[0:64, :])
    return nc


# ---------------------------------------------------------------- host side
def _prep_shared(enc_Wih, enc_Whh, enc_bih, enc_bhh,
                 dec_Wih, dec_Whh, dec_bih, dec_bhh, Wz, bz):
    bf = ml_dtypes.bfloat16
    f32 = np.float32

    def tobf(a):
        return np.ascontiguousarray(np.asarray(a, f32)).astype(bf)

    wihT = tobf(enc_Wih.T.reshape(I, 3 * H)).reshape(4, 128, 3 * H)
    whhT = tobf(enc_Whh.T).reshape(8, 128, 3 * H)
    wcomb = np.concatenate(
        [dec_Wih[: 2 * H] + dec_Whh[: 2 * H], dec_Wih[2 * H :], dec_Whh[2 * H :]], 0
    )
    wcombT = tobf(wcomb.T).reshape(8, 128, 4 * H)
    wzT = tobf(np.asarray(Wz, f32).T).reshape(8, 128, I)

    def stack_bias(bias4h):
        # [128, 4, 512]: rows 0:64 <- gate cols 0:512, rows 64:128 <- 512:1024
        out = np.zeros((128, 4, 512), f32)
        for gidx in range(4):
            seg = bias4h[gidx * H : (gidx + 1) * H]
            out[0:64, gidx, :] = seg[None, 0:512]
            out[64:128, gidx, :] = seg[None, 512:1024]
        return tobf(out)

    be = np.concatenate([np.asarray(enc_bih, f32)[: 2 * H]
                         + np.asarray(enc_bhh, f32)[: 2 * H],
                         np.asarray(enc_bih, f32)[2 * H :],
                         np.asarray(enc_bhh, f32)[2 * H :]])
    bd = np.concatenate([np.asarray(dec_bih, f32)[: 2 * H]
                         + np.asarray(dec_bhh, f32)[: 2 * H],
                         np.asarray(dec_bih, f32)[2 * H :],
                         np.asarray(dec_bhh, f32)[2 * H :]])

    def rep(row):
        return np.broadcast_to(np.asarray(row, f32)[None, :], (128, row.shape[0]))

    iden2 = np.concatenate([np.eye(64, dtype=f32)] * 2, axis=0)
    return {
        "wihT": wihT, "whhT": whhT, "wcombT": wcombT, "wzT": wzT,
        "bias_enc": stack_bias(be), "bias_dec": stack_bias(bd),
        "bz_rep": tobf(rep(np.asarray(bz, f32))),
        "behn_rep": tobf(rep(be[3 * H :])),
        "bdz_rep": tobf(rep(bd[H : 2 * H])),
        "sstat": np.full((128, 64), 1.0 / 128, f32).astype(bf),
        "iden2": tobf(iden2),
        "h0T": np.full((128, 512), 0.1, f32).astype(bf),
    }


def kernel(x, enc_Wih, enc_Whh, enc_bih, enc_bhh,
           dec_Wih, dec_Whh, dec_bih, dec_bhh, Wz, bz, n_steps=T):
    x = np.asarray(x, np.float32)
    shared = _prep_shared(enc_Wih, enc_Whh, enc_bih, enc_bhh,
                          dec_Wih, dec_Whh, dec_bih, dec_bhh, Wz, bz)
    in_maps = []
    for c in range(N_CORES):
        xc = x[c * BL : (c + 1) * BL, :n_steps]  # [BL, n_steps, I]
        xT = np.ascontiguousarray(xc.transpose(1, 2, 0)).reshape(n_steps, 4, 128, BL)
        in_maps.append({"xT": xT.astype(ml_dtypes.bfloat16), **shared})

    nc = build_nc(n_steps)
    _split_multi_waits(nc)

    trace = bool(int(os.environ.get("GRU_TRACE", "0")))
    if trace:
        _install_ntff_hook()
    res = bass_utils.run_bass_kernel_spmd(
        nc, in_maps, core_ids=list(range(N_CORES)), trace=trace
    )
    if trace and res.exec_time_ns is not None:
        print(f"HW exec time: {res.exec_time_ns} ns")
    out = np.concatenate([res.results[c]["z"] for c in range(N_CORES)], axis=0)
    return out


# revision 20
# speedup vs baseline: 1.0714x; 1.0714x over previous
"""GRU autoencoder Trainium2 kernel (bf16, seedless/no-inject redesign).

Data-parallel over batch: 8 cores x 64 rows. All gate matmuls are bf16
column-tile duos (tile_position (0,0)/(0,64)): each [128,512] PSUM tile
holds both 512-col halves of one gate. PSUM bias seeds, the PE inject,
and the z/n transposes of the old design are gone:
  - biases are added by DVE tensor_tensor (PSUM + stacked-bias tile),
  - n-preact sum (in + r*hn) runs on DVE,
  - the h-update runs in NORMAL space (h' = n + z*(h-n)) on 16-bit DVE
    ops, and only h' is transposed back (8 row-paired N=64 matmuls).
Decoder emits the z-gate last so the exposed post-matmul chain is just
zpre-add -> sigmoid -> e=z*d -> h'=n+e; zout duos fill the PE during it.
Hidden state is carried in both layouts: h_norm (stacked [128,512]) and
hT (hT[klo, 64*khi+b] = h[b, 128*khi+klo]).
"""
import os
import sys
import types

import ml_dtypes
import numpy as np

import concourse.bass as bass
import concourse.mybir as mybir
import concourse.tile as tile
from concourse import bass_utils

F32 = mybir.dt.float32
BF16 = mybir.dt.bfloat16
AF = mybir.ActivationFunctionType
OP = mybir.AluOpType

N_CORES = 8
B, T, I, H = 512, 128, 512, 1024
BL = B // N_CORES  # 64


# ---------------------------------------------------------------- fixups
def _split_multi_waits(nc, max_waits=1):
    """This walrus build allows only one sync-wait per instruction; hoist
    excess waits onto preceding NoOps (same engine, so semantics hold)."""
    for f in nc.m.functions:
        for blk in f.blocks:
            insts = blk.instructions
            if not any(
                i.sync_info is not None
                and i.sync_info.on_wait
                and len(i.sync_info.on_wait) > max_waits
                for i in insts
            ):
                continue
            new = []
            for inst in insts:
                si = inst.sync_info
                if si is not None and si.on_wait and len(si.on_wait) > max_waits:
                    waits = list(si.on_wait)
                    extra, keep = waits[:-max_waits], waits[-max_waits:]
                    for cs in range(0, len(extra), max_waits):
                        nop = mybir.InstNoOp(
                            name=nc.get_next_instruction_name(),
                            engine=inst.engine,
                            ins=[],
                            outs=[],
                            sync_info=mybir.SyncInfo(
                                on_wait=extra[cs : cs + max_waits], on_update=[]
                            ),
                        )
                        nc.register_instruction(nop)
                        new.append(nop)
                    si.on_wait = keep
                new.append(inst)
            insts[:] = new


def _install_ntff_hook():
    if "antenv.axon_hooks" in sys.modules:
        return True
    mod = types.ModuleType("antenv.axon_hooks")
    state = {"hook": None}
    mod.set_axon_ntff_profile_hook = lambda h: state.__setitem__("hook", h)
    mod.get_axon_ntff_profile_hook = lambda: state["hook"]
    sys.modules["antenv.axon_hooks"] = mod
    try:
        import antenv

        antenv.axon_hooks = mod
        from trn_agent_boot.trn_boot import _ntff_profile_via_ctypes

        hook = _ntff_profile_via_ctypes("/opt/axon/libaxon_pjrt.so")
        if hook is None:
            return False
        mod.set_axon_ntff_profile_hook(hook)
        return True
    except Exception:
        return False


# ---------------------------------------------------------------- program
def build_nc(n_steps=T):
    nc = bass.Bass("TRN2", target_bir_lowering=False, debug=False, num_devices=N_CORES)

    xT_d = nc.dram_tensor("xT", [n_steps, 4, 128, BL], BF16, kind="ExternalInput").ap()
    wih_d = nc.dram_tensor("wihT", [4, 128, 3 * H], BF16, kind="ExternalInput").ap()
    whh_d = nc.dram_tensor("whhT", [8, 128, 3 * H], BF16, kind="ExternalInput").ap()
    wcb_d = nc.dram_tensor("wcombT", [8, 128, 4 * H], BF16, kind="ExternalInput").ap()
    wz_d = nc.dram_tensor("wzT", [8, 128, I], BF16, kind="ExternalInput").ap()
    be_d = nc.dram_tensor("bias_enc", [128, 4, 512], BF16, kind="ExternalInput").ap()
    bd_d = nc.dram_tensor("bias_dec", [128, 4, 512], BF16, kind="ExternalInput").ap()
    bz_d = nc.dram_tensor("bz_rep", [128, I], BF16, kind="ExternalInput").ap()
    id_d = nc.dram_tensor("iden2", [128, 64], BF16, kind="ExternalInput").ap()
    ss_d = nc.dram_tensor("sstat", [128, 64], BF16, kind="ExternalInput").ap()
    bhe_d = nc.dram_tensor("behn_rep", [128, H], BF16, kind="ExternalInput").ap()
    bdz_d = nc.dram_tensor("bdz_rep", [128, H], BF16, kind="ExternalInput").ap()
    h0_d = nc.dram_tensor("h0T", [128, 512], BF16, kind="ExternalInput").ap()
    z_d = nc.dram_tensor("z", [BL, n_steps, I], F32, kind="ExternalOutput").ap()

    # gate -> column offset (PyTorch order r,z,n); bias slot index matches
    C0 = {"r": 0, "z": H, "in": 2 * H, "hn": 3 * H}
    BSLOT = {"r": 0, "z": 1, "in": 2, "hn": 3}

    with tile.TileContext(nc) as tc:
        with (
            tc.tile_pool(name="wgt", bufs=1) as wgt,
            tc.tile_pool(name="cst", bufs=1) as cst,
            tc.tile_pool(name="hst", bufs=5) as hst,
            tc.tile_pool(name="hnm", bufs=3) as hnm,
            tc.tile_pool(name="xts", bufs=4) as xts,
            tc.tile_pool(name="gsb", bufs=2) as gsb,
            tc.tile_pool(name="tmp", bufs=2) as tmpp,
            tc.tile_pool(name="zo", bufs=2) as zop,
            tc.tile_pool(name="ps", bufs=8, space="PSUM") as ps,
        ):
            iden2 = cst.tile([128, 64], BF16)
            nc.sync.dma_start(iden2[:], id_d[:])
            sstat = cst.tile([128, 64], BF16)
            nc.sync.dma_start(sstat[:], ss_d[:])
            bias_enc = cst.tile([128, 4, 512], BF16)
            nc.sync.dma_start(bias_enc[:], be_d[:])
            behn_rep = cst.tile([128, H], BF16)
            nc.sync.dma_start(behn_rep[:], bhe_d[:])
            bias_dec = cst.tile([128, 4, 512], BF16)
            bdz_rep = cst.tile([128, H], BF16)
            bz_rep = cst.tile([128, I], BF16)
            nc.sync.dma_start(bz_rep[:], bz_d[:])
            hT = hst.tile([128, 512], BF16, tag="h")
            nc.sync.dma_start(hT[:], h0_d[:])
            hN = hnm.tile([128, 512], BF16, tag="hn")
            nc.sync.dma_start(hN[:], h0_d[:])

            # encoder-critical DMAs first; decoder weights follow and overlap
            # with encoder compute
            xt_tiles = {}
            wih = wgt.tile([128, 4, 3 * H], BF16)
            for k in range(4):
                nc.sync.dma_start(wih[:, k, :], wih_d[k])
            for t in range(min(3, n_steps)):
                xt_tiles[t] = xts.tile([128, 4, BL], BF16, tag="x", name=f"xt{t}")
                for k in range(4):
                    nc.sync.dma_start(xt_tiles[t][:, k, :], xT_d[t, k])
            whh = wgt.tile([128, 8, 3 * H], BF16)
            for k in range(8):
                eng = nc.sync if k % 2 == 0 else nc.scalar
                eng.dma_start(whh[:, k, :], whh_d[k])
            nc.scalar.dma_start(bias_dec[:], bd_d[:])
            nc.scalar.dma_start(bdz_rep[:], bdz_d[:])
            wcb = wgt.tile([128, 8, 4 * H], BF16)
            for k in range(8):
                nc.scalar.dma_start(wcb[:, k, :], wcb_d[k])
            wz = wgt.tile([128, 8, I], BF16)
            for k in range(8):
                nc.scalar.dma_start(wz[:, k, :], wz_d[k])

            def dup(t, stat, mov_lo, mov_hi, start, stop):
                """One column-tile duo: two concurrent M=64 matmuls sharing
                the stationary operand."""
                nc.tensor.matmul(t[0:64, :], stat, mov_lo, start=start,
                                 stop=stop, tile_position=(0, 0),
                                 skip_group_check=True)
                nc.tensor.matmul(t[64:128, :], stat, mov_hi, start=start,
                                 stop=stop, tile_position=(0, 64),
                                 skip_group_check=True)

            def emit_gi(g, xt, gates, ks=range(4), start=False):
                for gate in gates:
                    c0 = C0[gate]
                    t = g[gate]
                    for k in ks:
                        dup(t, xt[:, k, :], wih[:, k, c0 : c0 + 512],
                            wih[:, k, c0 + 512 : c0 + 1024],
                            start=(start and k == 0), stop=False)

            # k-order matches hT'-sub completion: sub1 covers hT cols
            # 0-255 (khi 0-3)... but transpose halves land as (j01 -> khi
            # {0,1,4,5}); with the two subs covering cols 0-255 / 256-511,
            # (0,1,2,3,...) waits sub1 then sub2.  Keep natural order.
            # k-order follows hT'-sub completion: half A of the w-
            # transpose covers hT cols 0-127 & 256-383 (k 0,1,4,5), half B
            # the rest -- so gh starts as soon as half A lands.
            GH_KS = (0, 1, 4, 5, 2, 3, 6, 7)

            def emit_gh(g, w, gates, hTc, start=False, stop=True, c0map=None):
                for gate in gates:
                    c0 = (c0map or C0)[gate]
                    t = g[gate]
                    for i, k in enumerate(GH_KS):
                        hs = hTc[:, 64 * k : 64 * k + 64]
                        dup(t, hs, w[:, k, c0 : c0 + 512],
                            w[:, k, c0 + 512 : c0 + 1024],
                            start=(start and i == 0), stop=(stop and i == 7))

            def seed(t, rep, start=True):
                dup(t, sstat[:, :], rep[:, 0:512], rep[:, 512:1024],
                    start=start, stop=False)

            def transpose_h(src_sb, pT0, pT8):
                """Row-paired transpose of stacked h' [128,512]: pT0 gets
                h-dims 0-511 (hT cols 0-255 layout), pT8 the hi half. Two
                separate PSUM banks: the row-paired matmuls write the same
                partitions concurrently, so they must not share a bank."""
                for j in range(4):
                    nc.tensor.matmul(
                        pT0[:, 64 * j : 64 * j + 64],
                        src_sb[0:64, 128 * j : 128 * j + 128],
                        iden2[0:64, :], start=True, stop=True,
                        tile_position=(0, 0), skip_group_check=True,
                    )
                    nc.tensor.matmul(
                        pT8[:, 64 * j : 64 * j + 64],
                        src_sb[64:128, 128 * j : 128 * j + 128],
                        iden2[64:128, :], start=True, stop=True,
                        tile_position=(64, 0), skip_group_check=True,
                    )

            def new_gates(t_id, gates):
                g = {}
                for gate in gates:
                    g[gate] = ps.tile([128, 512], F32, tag="ps", bufs=5,
                                      name=f"p{gate}{t_id}")
                return g

            def step_tail(t_id, g, bias, dec_order):
                """DVE/ACT tail with the w-update: w = (z-1)*(n-h), so
                h' = h - w in both layouts.  The n/z chain after the last
                gate is split into column halves so transposes and hT-subs
                pipeline with it."""
                rpre = tmpp.tile([128, 512], BF16, tag="rpre", name=f"rp{t_id}")
                rhn = tmpp.tile([128, 512], BF16, tag="rhn", name=f"rh{t_id}")
                inb = tmpp.tile([128, 512], BF16, tag="inb", name=f"ib{t_id}")
                npre = tmpp.tile([128, 512], BF16, tag="npre", name=f"np{t_id}")
                v = tmpp.tile([128, 512], BF16, tag="v", name=f"v{t_id}")
                w = tmpp.tile([128, 512], BF16, tag="w", name=f"w{t_id}")
                z_sb = gsb.tile([128, 512], BF16, tag="z", name=f"z{t_id}")
                r_sb = gsb.tile([128, 512], BF16, tag="r", name=f"r{t_id}")
                n_sb = gsb.tile([128, 512], BF16, tag="n", name=f"n{t_id}")
                LO, HI = slice(0, 256), slice(256, 512)

                def badd(out, gate):
                    s = BSLOT[gate]
                    nc.vector.tensor_add(out[:], g[gate][:], bias[:, s, :])

                if not dec_order:
                    # encoder: z,r early; hn seeded -> rhn reads PSUM direct
                    zpre = tmpp.tile([128, 512], BF16, tag="zpre",
                                     name=f"zp{t_id}")
                    badd(zpre, "z")
                    nc.scalar.activation(z_sb[:], zpre[:], AF.Sigmoid)
                    badd(rpre, "r")
                    nc.scalar.activation(r_sb[:], rpre[:], AF.Sigmoid)
                    badd(inb, "in")
                    nc.vector.tensor_mul(rhn[:], r_sb[:], g["hn"][:])
                    nc.vector.tensor_add(npre[:], inb[:], rhn[:])
                    nc.scalar.activation(n_sb[:], npre[:], AF.Tanh)
                    nc.vector.tensor_sub(v[:], n_sb[:], hN[:])
                    nc.vector.scalar_tensor_tensor(
                        w[:], z_sb[:], 1.0, v[:], OP.subtract, OP.mult)
                else:
                    # decoder: r,hn,in done; z (seeded) completes last and
                    # its sigmoid reads PSUM directly, by halves
                    hnb = tmpp.tile([128, 512], BF16, tag="hnb",
                                    name=f"hb{t_id}")
                    badd(rpre, "r")
                    nc.scalar.activation(r_sb[:], rpre[:], AF.Sigmoid)
                    badd(hnb, "hn")
                    nc.vector.tensor_mul(rhn[:], r_sb[:], hnb[:])
                    badd(inb, "in")
                    nc.vector.tensor_add(npre[:], inb[:], rhn[:])
                    nc.scalar.activation(n_sb[:], npre[:], AF.Tanh)
                    nc.vector.tensor_sub(v[:], n_sb[:], hN[:])
                    nc.scalar.activation(z_sb[:], g["z"][:], AF.Sigmoid)
                    nc.vector.scalar_tensor_tensor(
                        w[:], z_sb[:], 1.0, v[:], OP.subtract, OP.mult)
                return w

            def finish_h(t_id, w):
                """Per-half: transpose w (row-paired matmul duos), then
                hT' = hT - wT via DVE subs reading the transpose PSUM, so
                gh k(0,1,4,5) unblocks after half A.  h'_norm = h_norm - w."""
                nonlocal hT, hN
                pT0 = ps.tile([128, 512], F32, tag="pt0", bufs=1,
                              name=f"pT0_{t_id}")
                pT8 = ps.tile([128, 512], F32, tag="pt8", bufs=1,
                              name=f"pT8_{t_id}")
                hT_new = hst.tile([128, 512], BF16, tag="h", name=f"h{t_id}")
                for half in (0, 1):
                    for j in (2 * half, 2 * half + 1):
                        nc.tensor.matmul(
                            pT0[:, 64 * j : 64 * j + 64],
                            w[0:64, 128 * j : 128 * j + 128],
                            iden2[0:64, :], start=True, stop=True,
                            tile_position=(0, 0), skip_group_check=True,
                        )
                        nc.tensor.matmul(
                            pT8[:, 64 * j : 64 * j + 64],
                            w[64:128, 128 * j : 128 * j + 128],
                            iden2[64:128, :], start=True, stop=True,
                            tile_position=(64, 0), skip_group_check=True,
                        )
                    s = slice(128 * half, 128 * half + 128)
                    nc.vector.tensor_sub(hT_new[:, s], hT[:, s], pT0[:, s])
                    s2 = slice(256 + 128 * half, 256 + 128 * half + 128)
                    nc.vector.tensor_sub(hT_new[:, s2], hT[:, s2], pT8[:, s])
                hN_new = hnm.tile([128, 512], BF16, tag="hn", name=f"hN{t_id}")
                nc.gpsimd.tensor_sub(hN_new[:], hN[:], w[:])
                hT = hT_new
                hN = hN_new
                return hT_new

            # ================= encoder =================
            cur = new_gates(0, ("z", "r", "in"))
            emit_gi(cur, xt_tiles[0], ("z", "r", "in"), start=True)

            for t in range(n_steps):
                if t + 3 < n_steps:
                    xt_tiles[t + 3] = xts.tile([128, 4, BL], BF16, tag="x",
                                               name=f"xt{t+3}")
                    for k in range(4):
                        nc.sync.dma_start(xt_tiles[t + 3][:, k, :], xT_d[t + 3, k])
                cur["hn"] = ps.tile([128, 512], F32, tag="ps", bufs=5,
                                    name=f"phn{t}")
                seed(cur["hn"], behn_rep)
                emit_gh(cur, whh, ("z", "r"), hT)
                emit_gh(cur, whh, ("hn",), hT, c0map={"hn": 2 * H})
                g = cur
                w = step_tail(t, g, bias_enc, dec_order=False)
                if t + 1 < n_steps:
                    cur = new_gates(t + 1, ("z", "r", "in"))
                    emit_gi(cur, xt_tiles[t + 1], ("z", "r"), start=True)
                    emit_gi(cur, xt_tiles[t + 1], ("in",), ks=(0,), start=True)
                finish_h(t, w)
                if t + 1 < n_steps:
                    # held-back duos fill the PE while the hT-subs drain
                    emit_gi(cur, xt_tiles[t + 1], ("in",), ks=(1, 2, 3))
                xt_tiles.pop(t, None)

            # ================= decoder =================
            # z-output pair p covers steps (p, p+1) using entries[p]=H_p and
            # entries[p+1]; its 8 duos spread over steps p+2, p+3 as PE
            # fillers during the exposed tail.
            assert n_steps == 1 or n_steps % 2 == 0
            zfill_ps = [None]
            entries = {}

            def emit_zpair(p, ks, finish):
                hA, hB = entries[p], entries[p + 1]
                if zfill_ps[0] is None:
                    zfill_ps[0] = ps.tile([128, 512], F32, tag="zo", bufs=1,
                                          name=f"pzo{p}")
                tz = zfill_ps[0]
                for k in ks:
                    nc.tensor.matmul(tz[0:64, :], hA[:, 64 * k : 64 * k + 64],
                                     wz[:, k, :], start=(k == 0),
                                     stop=(k == 7), tile_position=(0, 0),
                                     skip_group_check=True)
                    nc.tensor.matmul(tz[64:128, :], hB[:, 64 * k : 64 * k + 64],
                                     wz[:, k, :], start=(k == 0),
                                     stop=(k == 7), tile_position=(0, 64),
                                     skip_group_check=True)
                if finish:
                    zo_sb = zop.tile([128, 512], F32, tag="zo", name=f"zo{p}")
                    nc.vector.tensor_add(zo_sb[:], tz[:], bz_rep[:])
                    nc.sync.dma_start(z_d[:, p, :], zo_sb[0:64, :])
                    nc.sync.dma_start(z_d[:, p + 1, :], zo_sb[64:128, :])
                    zfill_ps[0] = None

            for t in range(n_steps):
                entries[t - 1] = hT  # hT entering step t is H_{t-1}
                cur = new_gates(1000 + t, ("r", "hn", "in", "z"))
                seed(cur["z"], bdz_rep)
                emit_gh(cur, wcb, ("r", "hn", "in"), hT, start=True)
                emit_gh(cur, wcb, ("z",), hT)
                g = cur
                w = step_tail(1000 + t, g, bias_dec, dec_order=True)
                # zout fillers run while the z-chain drains
                if t >= 2 and t % 2 == 0:
                    emit_zpair(t - 2, ks=range(0, 4), finish=False)
                elif t >= 3 and t % 2 == 1:
                    emit_zpair(t - 3, ks=range(4, 8), finish=True)
                finish_h(1000 + t, w)
            entries[n_steps - 1] = hT

            if n_steps >= 2:
                emit_zpair(n_steps - 2, ks=range(0, 8), finish=True)
            elif n_steps == 1:
                # single step: both column-duo halves compute H_0 @ Wz
                tz = ps.tile([128, 512], F32, tag="zo", bufs=1, name="pzo0")
                for k in range(8):
                    dup(tz, entries[0][:, 64 * k : 64 * k + 64], wz[:, k, :],
                        wz[:, k, :], start=(k == 0), stop=(k == 7))
                zo_sb = zop.tile([128, 512], F32, tag="zo", name="zo0")
                nc.vector.tensor_add(zo_sb[:], tz[:], bz_rep[:])
                nc.sync.dma_start(z_d[:, 0, :], zo_sb[0:64, :])
    return nc


# ---------------------------------------------------------------- host side
def _prep_shared(enc_Wih, enc_Whh, enc_bih, enc_bhh,
                 dec_Wih, dec_Whh, dec_bih, dec_bhh, Wz, bz):
    bf = ml_dtypes.bfloat16
    f32 = np.float32

    def tobf(a):
        return np.ascontiguousarray(np.asarray(a, f32)).astype(bf)

    wihT = tobf(enc_Wih.T.reshape(I, 3 * H)).reshape(4, 128, 3 * H)
    whhT = tobf(enc_Whh.T).reshape(8, 128, 3 * H)
    wcomb = np.concatenate(
        [dec_Wih[: 2 * H] + dec_Whh[: 2 * H], dec_Wih[2 * H :], dec_Whh[2 * H :]], 0
    )
    wcombT = tobf(wcomb.T).reshape(8, 128, 4 * H)
    wzT = tobf(np.asarray(Wz, f32).T).reshape(8, 128, I)

    def stack_bias(bias4h):
        # [128, 4, 512]: rows 0:64 <- gate cols 0:512, rows 64:128 <- 512:1024
        out = np.zeros((128, 4, 512), f32)
        for gidx in range(4):
            seg = bias4h[gidx * H : (gidx + 1) * H]
            out[0:64, gidx, :] = seg[None, 0:512]
            out[64:128, gidx, :] = seg[None, 512:1024]
        return tobf(out)

    be = np.concatenate([np.asarray(enc_bih, f32)[: 2 * H]
                         + np.asarray(enc_bhh, f32)[: 2 * H],
                         np.asarray(enc_bih, f32)[2 * H :],
                         np.asarray(enc_bhh, f32)[2 * H :]])
    bd = np.concatenate([np.asarray(dec_bih, f32)[: 2 * H]
                         + np.asarray(dec_bhh, f32)[: 2 * H],
                         np.asarray(dec_bih, f32)[2 * H :],
                         np.asarray(dec_bhh, f32)[2 * H :]])

    def rep(row):
        return np.broadcast_to(np.asarray(row, f32)[None, :], (128, row.shape[0]))

    iden2 = np.concatenate([np.eye(64, dtype=f32)] * 2, axis=0)
    return {
        "wihT": wihT, "whhT": whhT, "wcombT": wcombT, "wzT": wzT,
        "bias_enc": stack_bias(be), "bias_dec": stack_bias(bd),
        "bz_rep": tobf(rep(np.asarray(bz, f32))),
        "behn_rep": tobf(rep(be[3 * H :])),
        "bdz_rep": tobf(rep(bd[H : 2 * H])),
        "sstat": np.full((128, 64), 1.0 / 128, f32).astype(bf),
        "iden2": tobf(iden2),
        "h0T": np.full((128, 512), 0.1, f32).astype(bf),
    }


def kernel(x, enc_Wih, enc_Whh, enc_bih, enc_bhh,
           dec_Wih, dec_Whh, dec_bih, dec_bhh, Wz, bz, n_steps=T):
    x = np.asarray(x, np.float32)
    shared = _prep_shared(enc_Wih, enc_Whh, enc_bih, enc_bhh,
                          dec_Wih, dec_Whh, dec_bih, dec_bhh, Wz, bz)
    in_maps = []
    for c in range(N_CORES):
        xc = x[c * BL : (c + 1) * BL, :n_steps]  # [BL, n_steps, I]
        xT = np.ascontiguousarray(xc.transpose(1, 2, 0)).reshape(n_steps, 4, 128, BL)
        in_maps.append({"xT": xT.astype(ml_dtypes.bfloat16), **shared})

    nc = build_nc(n_steps)
    _split_multi_waits(nc)

    trace = bool(int(os.environ.get("GRU_TRACE", "0")))
    if trace:
        _install_ntff_hook()
    res = bass_utils.run_bass_kernel_spmd(
        nc, in_maps, core_ids=list(range(N_CORES)), trace=trace
    )
    if trace and res.exec_time_ns is not None:
        print(f"HW exec time: {res.exec_time_ns} ns")
    out = np.concatenate([res.results[c]["z"] for c in range(N_CORES)], axis=0)
    return out
